# revision 1
# baseline (speedup 1.0000x reference)
"""FP8 linear kernel for Trainium2, 8 NeuronCores.

y = (quant_e4m3fn(x) @ W.T) * (x_inv_scale * w_scale), output bf16.

Sharding: 2 token-halves x 4 out_feature-quarters (tensor parallel on
out_features per the hint, plus 2-way data parallel on tokens).

Exactness strategy: TRN fp8_e4m3 saturates at +-240 (vs OCP e4m3fn's
448), so both operands are staged/quantized at HALF scale (values <=
224), where the two formats agree bit-for-bit, and the dequant factor
carries the compensating 4x. Halving is exact for fp8 normals, so the
kernel reproduces the reference quantization exactly (modulo the
subnormal tail and fp32 summation order).

The global activation amax is computed cooperatively: each core reduces
1/8 of x locally, then an AllReduce(max) collective merges the 8 partial
maxima on-device.
"""

import os
import sys

for _p in ("/opt/trn_rl_repo", "/root/.axon_site/_ro/trn_rl_repo"):
    if os.path.isdir(_p) and _p not in sys.path:
        sys.path.insert(0, _p)

import ml_dtypes
import numpy as np

import concourse.bass as bass
import concourse.bass_isa as bass_isa
import concourse.mybir as mybir
import concourse.tile as tile
from concourse import bacc
from concourse.bass import ds, ts
from concourse.bass_utils import run_bass_kernel_spmd

F32 = mybir.dt.float32
BF16 = mybir.dt.bfloat16
FP8 = mybir.dt.float8e4
FP8_NP = mybir.dt.np(FP8)
E4M3FN = ml_dtypes.float8_e4m3fn

TOKENS, DIN, DOUT = 8192, 4096, 14336
P = 128
KO = DIN // P               # 32 k-subtiles of 128
N_CORES = 8
TOK_WAYS, OF_WAYS = 2, 4    # sharding grid
T_SH = TOKENS // TOK_WAYS   # 4096 tokens per core
OF_SH = DOUT // OF_WAYS     # 3584 out features per core
TT = T_SH // P              # 32 token tiles per core
AT = TT // OF_WAYS          # 8 amax tiles per core (1/8 of x total)
NB = 448                    # psum bank free dim; 4 banks = 1792 = OF_SH/2
OF_HALF = OF_SH // 2        # 1792

# 'doublerow' (fast fp8, ~1e-4 extra accum noise) or 'plain' (exact fp8)
PERF = os.environ.get("FP8LIN_PERF", "doublerow")

_CACHE = {}


def _build_module():
    key = ("module", PERF)
    if key in _CACHE:
        return _CACHE[key]

    nc = bacc.Bacc(None, target_bir_lowering=False, debug=True, num_devices=N_CORES)
    xh = nc.declare_dram_parameter("xh", [TT, P, KO, P], F32, isOutput=False)
    xa = nc.declare_dram_parameter("xa", [AT, P, KO, P], F32, isOutput=False)
    w3 = nc.declare_dram_parameter("w3", [P, KO, OF_SH], FP8, isOutput=False)
    ws = nc.declare_dram_parameter("ws", [P, 1], F32, isOutput=False)
    y = nc.declare_dram_parameter("y", [T_SH, OF_SH], BF16, isOutput=True)
    cc_in = nc.dram_tensor("cc_in", [1], F32)
    cc_out = nc.dram_tensor("cc_out", [1], F32, addr_space="Shared")

    dr = mybir.MatmulPerfMode.DoubleRow if PERF == "doublerow" else None

    with tile.TileContext(nc) as tc:
        with (
            tc.tile_pool(name="const", bufs=1) as const,
            tc.tile_pool(name="work", bufs=3) as work,
            tc.tile_pool(name="xqp", bufs=2) as xqp,
            tc.tile_pool(name="psum", bufs=2, space="PSUM") as psum,
        ):
            # resident weight [128, KO, OF_SH] fp8 (14.3 MB), halves for
            # DMA-queue parallelism
            wres = const.tile([P, KO, OF_SH], FP8)
            for i in range(4):
                nc.sync.dma_start(
                    out=wres[:, ts(i, KO // 4), :], in_=w3[:, ts(i, KO // 4), :]
                )
            wssb = const.tile([P, 1], F32)
            nc.sync.dma_start(out=wssb[:], in_=ws[:])

            # ---- phase A: global amax of x ----
            partials = const.tile([P, AT], F32)
            for i in range(AT):
                xt = work.tile([P, KO, P], F32, tag="xf")
                nc.sync.dma_start(out=xt[:], in_=xa[i])
                nc.vector.tensor_reduce(
                    partials[:, i : i + 1],
                    xt[:],
                    axis=mybir.AxisListType.XY,
                    op=mybir.AluOpType.max,
                    apply_absolute_value=True,
                )
            loc = const.tile([P, 1], F32)
            nc.vector.tensor_reduce(
                loc[:], partials[:], axis=mybir.AxisListType.X, op=mybir.AluOpType.max
            )
            allp = const.tile([P, 1], F32)
            nc.gpsimd.partition_all_reduce(
                allp[:], loc[:], channels=P, reduce_op=bass_isa.ReduceOp.max
            )
            nc.sync.dma_start(out=cc_in[:], in_=allp[0, :])
            nc.gpsimd.collective_compute(
                "AllReduce",
                mybir.AluOpType.max,
                replica_groups=[list(range(N_CORES))],
                ins=[cc_in[:]],
                outs=[cc_out[:]],
            )
            g1 = const.tile([1, 1], F32)
            nc.sync.dma_start(out=g1[:], in_=cc_out[:])
            nc.vector.tensor_scalar_max(g1[:], g1[:], 1e-12)
            gb = const.tile([P, 1], F32)
            nc.gpsimd.partition_broadcast(gb[:], g1[:])
            # quant multiplier 224/amax == (448/amax)/2 exactly
            recip = const.tile([P, 1], F32)
            nc.vector.reciprocal(recip[:], gb[:])
            qv = const.tile([P, 1], F32)
            nc.vector.tensor_scalar_mul(qv[:], recip[:], 224.0)
            # dequant multiplier amax*w_scale/112 == 4 * (amax/448) * w_scale
            mf = const.tile([P, 1], F32)
            nc.vector.tensor_mul(out=mf[:], in0=gb[:], in1=wssb[:])
            nc.vector.tensor_scalar_mul(mf[:], mf[:], 1.0 / 112.0)

            # ---- phase B: quantize + matmul + scaled eviction ----
            for t in range(TT):
                xf = work.tile([P, KO, P], F32, tag="xf")
                nc.sync.dma_start(out=xf[:], in_=xh[t])
                xq = xqp.tile([P, KO, P], FP8, tag="xq")
                nc.scalar.activation(
                    xq[:], xf[:], mybir.ActivationFunctionType.Copy, scale=qv[:]
                )
                for h2 in range(2):
                    ps = [
                        psum.tile([P, NB], F32, name=f"ps{i}") for i in range(4)
                    ]
                    if dr is not None:
                        for k2 in range(KO // 2):
                            lhsT = xq[:, 2 * k2 : 2 * k2 + 2, :]
                            for nb in range(4):
                                nc.tensor.matmul(
                                    ps[nb][:],
                                    lhsT,
                                    wres[:, 2 * k2 : 2 * k2 + 2,
                                         ds(h2 * OF_HALF + nb * NB, NB)],
                                    start=(k2 == 0),
                                    stop=(k2 == KO // 2 - 1),
                                    perf_mode=dr,
                                )
                    else:
                        for k in range(KO):
                            lhsT = xq[:, k, :]
                            for nb in range(4):
                                nc.tensor.matmul(
                                    ps[nb][:],
                                    lhsT,
                                    wres[:, k, ds(h2 * OF_HALF + nb * NB, NB)],
                                    start=(k == 0),
                                    stop=(k == KO - 1),
                                )
                    yt = work.tile([P, 4, NB], BF16, tag="yt")
                    for nb in range(4):
                        nc.vector.tensor_scalar_mul(yt[:, nb, :], ps[nb][:], mf[:])
                    nc.sync.dma_start(
                        out=y[ts(t, P), ds(h2 * OF_HALF, OF_HALF)],
                        in_=yt[:],
                    )

    nc.compile()
    _CACHE[key] = nc
    return nc


def _pack_inputs(x, weight, w_scale):
    """Host-side shard + layout packing. Returns in_maps for 8 cores."""
    x = np.asarray(x, dtype=np.float32)
    w_fp8fn = np.asarray(weight)
    if w_fp8fn.dtype != E4M3FN:
        w_fp8fn = w_fp8fn.view(E4M3FN) if w_fp8fn.itemsize == 1 else w_fp8fn.astype(E4M3FN)
    ws_val = float(np.asarray(w_scale, dtype=np.float32).reshape(()))

    # x packed per token-half: [TT, P, KO, P]; [t, p, ko, m] = x[h*T_SH + t*128 + m, ko*128 + p]
    halves = []
    for h in range(TOK_WAYS):
        xhalf = x[h * T_SH : (h + 1) * T_SH]
        packed = np.ascontiguousarray(
            xhalf.reshape(TT, P, KO, P).transpose(0, 3, 2, 1)
        )
        halves.append(packed)

    # W at half scale (exact for fp8 normals), TRN fp8 range
    w_half = (w_fp8fn.astype(np.float32) * 0.5).astype(E4M3FN)
    wqs = []
    for q in range(OF_WAYS):
        wq = w_half[q * OF_SH : (q + 1) * OF_SH]          # [OF_SH, DIN]
        w3 = np.ascontiguousarray(
            wq.T.reshape(KO, P, OF_SH).transpose(1, 0, 2)  # [P, KO, OF_SH]
        ).view(FP8_NP)
        wqs.append(w3)

    ws_arr = np.full((P, 1), ws_val, dtype=np.float32)

    in_maps = []
    for c in range(N_CORES):
        h, q = c // OF_WAYS, c % OF_WAYS
        in_maps.append(
            {
                "xh": halves[h],
                "xa": halves[h][q * AT : (q + 1) * AT],
                "w3": wqs[q],
                "ws": ws_arr,
            }
        )
    return in_maps


def _assemble(results):
    y = np.empty((TOKENS, DOUT), dtype=ml_dtypes.bfloat16)
    for c in range(N_CORES):
        h, q = c // OF_WAYS, c % OF_WAYS
        part = results[c]["y"]
        if part.dtype != ml_dtypes.bfloat16:
            part = part.view(ml_dtypes.bfloat16)
        y[h * T_SH : (h + 1) * T_SH, q * OF_SH : (q + 1) * OF_SH] = part
    return y


def kernel(x, weight, w_scale):
    nc = _build_module()
    in_maps = _pack_inputs(x, weight, w_scale)
    res = run_bass_kernel_spmd(nc, in_maps, list(range(N_CORES)))
    return _assemble(res.results)


# revision 14
# speedup vs baseline: 23927.2057x; 23927.2057x over previous
"""FP8 linear kernel for Trainium2, 8 NeuronCores.

y = (quant_e4m3fn(x) @ W.T) * (x_inv_scale * w_scale), output bf16.

Sharding: 2 token-halves x 4 out_feature-quarters (tensor parallel on
out_features per the hint, plus 2-way data parallel on tokens).

Exactness strategy: TRN fp8_e4m3 saturates at +-240 (vs OCP e4m3fn's
448), so both operands are staged/quantized at HALF scale (values <=
224), where the two formats agree bit-for-bit, and the dequant factor
carries the compensating 4x. Halving is exact for fp8 normals, so the
kernel reproduces the reference quantization exactly (modulo the
subnormal tail and fp32 summation order).

The global activation amax is computed cooperatively: each core reduces
1/8 of x locally, then an AllReduce(max) collective merges the 8 partial
maxima on-device.
"""

import contextlib
import os
import sys

for _p in ("/opt/trn_rl_repo", "/root/.axon_site/_ro/trn_rl_repo"):
    if os.path.isdir(_p) and _p not in sys.path:
        sys.path.insert(0, _p)

import ml_dtypes
import numpy as np

import concourse.bass as bass
import concourse.bass_isa as bass_isa
import concourse.mybir as mybir
import concourse.tile as tile
from concourse import bacc
from concourse.bass import ds, ts
from concourse.bass_utils import run_bass_kernel_spmd

F32 = mybir.dt.float32
BF16 = mybir.dt.bfloat16
FP8 = mybir.dt.float8e4
FP8_NP = mybir.dt.np(FP8)
E4M3FN = ml_dtypes.float8_e4m3fn

TOKENS, DIN, DOUT = 8192, 4096, 14336
P = 128
KO = DIN // P               # 32 k-subtiles of 128
N_CORES = 8
TOK_WAYS, OF_WAYS = 2, 4    # sharding grid
T_SH = TOKENS // TOK_WAYS   # 4096 tokens per core
OF_SH = DOUT // OF_WAYS     # 3584 out features per core
TT = T_SH // P              # 32 token tiles per core
AT = TT // OF_WAYS          # 8 amax tiles per core (1/8 of x total)
NB = 448                    # psum bank free dim; 4 banks = 1792 = OF_SH/2
OF_HALF = OF_SH // 2        # 1792

# 'doublerow' (fast fp8, ~1e-4 extra accum noise) or 'plain' (exact fp8)
PERF = os.environ.get("FP8LIN_PERF", "doublerow")
# benchmarking aid: skip the amax phase, use dummy constant scales
PHASE_A = os.environ.get("FP8LIN_PHASE_A", "on") == "on"

_CACHE = {}


STRUCT = os.environ.get("FP8LIN_STRUCT", "v1")


def _phase_b(nc, work, xqp, psum, xh, y, wres, qv, mf, dr):
    """Quantize + matmul + scaled eviction over all token tiles."""
    xf_bufs = 4 if STRUCT == "v4" else 3
    yt_bufs = 2 if STRUCT == "v4" else 3
    for t in range(TT):
        xf = work.tile([P, KO, P], F32, tag="xf", bufs=xf_bufs)
        nc.sync.dma_start(out=xf[:], in_=xh[t])
        xq = xqp.tile([P, KO, P], FP8, tag="xq")
        if STRUCT == "v5":
            nc.vector.tensor_scalar_mul(xq[:], xf[:], qv[:])
        else:
            nc.scalar.activation(
                xq[:], xf[:], mybir.ActivationFunctionType.Copy, scale=qv[:]
            )
        if STRUCT == "v3":
            # one 8-bank psum group; 8 MMs per stationary; evict stalls PE
            ps = [psum.tile([P, NB], F32, name=f"ps{i}", bufs=1) for i in range(8)]
            for k2 in range(KO // 2):
                lhsT = xq[:, 2 * k2 : 2 * k2 + 2, :]
                for nb in range(8):
                    nc.tensor.matmul(
                        ps[nb][:], lhsT, wres[:, 2 * k2 : 2 * k2 + 2, ds(nb * NB, NB)],
                        start=(k2 == 0), stop=(k2 == KO // 2 - 1), perf_mode=dr,
                    )
            yt = work.tile([P, 8, NB], BF16, tag="yt")
            for nb in range(8):
                nc.vector.tensor_scalar_mul(yt[:, nb, :], ps[nb][:], mf[:])
            nc.sync.dma_start(out=y[ts(t, P), :], in_=yt[:])
            continue
        for h2 in range(2):
            ps = [psum.tile([P, NB], F32, name=f"ps{i}") for i in range(4)]
            if dr is not None:
                if STRUCT == "v2":
                    # nb outer, k2 inner: no stationary sharing
                    for nb in range(4):
                        for k2 in range(KO // 2):
                            nc.tensor.matmul(
                                ps[nb][:],
                                xq[:, 2 * k2 : 2 * k2 + 2, :],
                                wres[:, 2 * k2 : 2 * k2 + 2,
                                     ds(h2 * OF_HALF + nb * NB, NB)],
                                start=(k2 == 0),
                                stop=(k2 == KO // 2 - 1),
                                perf_mode=dr,
                            )
                else:
                    for k2 in range(KO // 2):
                        lhsT = xq[:, 2 * k2 : 2 * k2 + 2, :]
                        for nb in range(4):
                            nc.tensor.matmul(
                                ps[nb][:],
                                lhsT,
                                wres[:, 2 * k2 : 2 * k2 + 2,
                                     ds(h2 * OF_HALF + nb * NB, NB)],
                                start=(k2 == 0),
                                stop=(k2 == KO // 2 - 1),
                                perf_mode=dr,
                            )
            else:
                for k in range(KO):
                    lhsT = xq[:, k, :]
                    for nb in range(4):
                        nc.tensor.matmul(
                            ps[nb][:],
                            lhsT,
                            wres[:, k, ds(h2 * OF_HALF + nb * NB, NB)],
                            start=(k == 0),
                            stop=(k == KO - 1),
                        )
            yt = work.tile([P, 4, NB], BF16, tag="yt", bufs=yt_bufs)
            for nb in range(4):
                nc.vector.tensor_scalar_mul(yt[:, nb, :], ps[nb][:], mf[:])
            nc.sync.dma_start(
                out=y[ts(t, P), ds(h2 * OF_HALF, OF_HALF)],
                in_=yt[:],
            )


def _build_module(reps=1, phase_a=None):
    if phase_a is None:
        phase_a = PHASE_A
    key = ("module", PERF, reps, phase_a, STRUCT)
    if key in _CACHE:
        return _CACHE[key]

    nc = bacc.Bacc(None, target_bir_lowering=False, debug=True, num_devices=N_CORES)
    xh = nc.declare_dram_parameter("xh", [TT, P, KO, P], F32, isOutput=False)
    xa = nc.declare_dram_parameter("xa", [AT, P, KO, P], F32, isOutput=False)
    w3 = nc.declare_dram_parameter("w3", [P, KO, OF_SH], FP8, isOutput=False)
    ws = nc.declare_dram_parameter("ws", [P, 1], F32, isOutput=False)
    y = nc.declare_dram_parameter("y", [T_SH, OF_SH], BF16, isOutput=True)
    cc_in = nc.dram_tensor("cc_in", [1], F32)
    cc_out = nc.dram_tensor("cc_out", [1], F32, addr_space="Shared")

    dr = mybir.MatmulPerfMode.DoubleRow if PERF == "doublerow" else None

    with tile.TileContext(nc) as tc:
        with (
            tc.tile_pool(name="const", bufs=1) as const,
            tc.tile_pool(name="work", bufs=3) as work,
            tc.tile_pool(name="xqp", bufs=2) as xqp,
            tc.tile_pool(name="psum", bufs=2, space="PSUM") as psum,
        ):
            # resident weight [128, KO, OF_SH] fp8 (14.3 MB)
            wres = const.tile([P, KO, OF_SH], FP8)
            for i in range(4):
                nc.sync.dma_start(
                    out=wres[:, ts(i, KO // 4), :], in_=w3[:, ts(i, KO // 4), :]
                )
            wssb = const.tile([P, 1], F32)
            nc.sync.dma_start(out=wssb[:], in_=ws[:])

            if phase_a:
                # ---- phase A: global amax of x ----
                partials = const.tile([P, AT], F32)
                for i in range(AT):
                    xt = work.tile(
                        [P, KO, P], F32, tag="xf",
                        bufs=4 if STRUCT == "v4" else 3,
                    )
                    nc.sync.dma_start(out=xt[:], in_=xa[i])
                    nc.vector.tensor_reduce(
                        partials[:, i : i + 1],
                        xt[:],
                        axis=mybir.AxisListType.XY,
                        op=mybir.AluOpType.max,
                        apply_absolute_value=True,
                    )
                loc = const.tile([P, 1], F32)
                nc.vector.tensor_reduce(
                    loc[:], partials[:], axis=mybir.AxisListType.X,
                    op=mybir.AluOpType.max,
                )
                allp = const.tile([P, 1], F32)
                nc.gpsimd.partition_all_reduce(
                    allp[:], loc[:], channels=P, reduce_op=bass_isa.ReduceOp.max
                )
                nc.sync.dma_start(out=cc_in[:], in_=allp[0, :])
                nc.gpsimd.collective_compute(
                    "AllReduce",
                    mybir.AluOpType.max,
                    replica_groups=[list(range(N_CORES))],
                    ins=[cc_in[:]],
                    outs=[cc_out[:]],
                )
                g1 = const.tile([1, 1], F32)
                nc.sync.dma_start(out=g1[:], in_=cc_out[:])
                nc.vector.tensor_scalar_max(g1[:], g1[:], 1e-12)
                gb = const.tile([P, 1], F32)
                nc.gpsimd.partition_broadcast(gb[:], g1[:])
                # quant multiplier 224/amax == (448/amax)/2 exactly
                recip = const.tile([P, 1], F32)
                nc.vector.reciprocal(recip[:], gb[:])
                qv = const.tile([P, 1], F32)
                nc.vector.tensor_scalar_mul(qv[:], recip[:], 224.0)
                # dequant multiplier amax*w_scale/112 == 4 * (amax/448) * w_scale
                mf = const.tile([P, 1], F32)
                nc.vector.tensor_mul(out=mf[:], in0=gb[:], in1=wssb[:])
                nc.vector.tensor_scalar_mul(mf[:], mf[:], 1.0 / 112.0)
            else:
                qv = const.tile([P, 1], F32)
                nc.vector.memset(qv[:], 0.125)
                mf = const.tile([P, 1], F32)
                nc.vector.memset(mf[:], 8.0)

            # ---- phase B (reps>1 loops it, for slope benchmarking only) ----
            loop_ctx = tc.For_i(0, reps, 1) if reps > 1 else contextlib.nullcontext()
            with loop_ctx:
                _phase_b(nc, work, xqp, psum, xh, y, wres, qv, mf, dr)

    nc.compile()
    _CACHE[key] = nc
    return nc


def _pack_inputs(x, weight, w_scale):
    """Host-side shard + layout packing. Returns in_maps for 8 cores."""
    x = np.asarray(x, dtype=np.float32)
    w_fp8fn = np.asarray(weight)
    if w_fp8fn.dtype != E4M3FN:
        w_fp8fn = w_fp8fn.view(E4M3FN) if w_fp8fn.itemsize == 1 else w_fp8fn.astype(E4M3FN)
    ws_val = float(np.asarray(w_scale, dtype=np.float32).reshape(()))

    # x packed per token-half: [TT, P, KO, P]; [t, p, ko, m] = x[h*T_SH + t*128 + m, ko*128 + p]
    halves = []
    for h in range(TOK_WAYS):
        xhalf = x[h * T_SH : (h + 1) * T_SH]
        packed = np.ascontiguousarray(
            xhalf.reshape(TT, P, KO, P).transpose(0, 3, 2, 1)
        )
        halves.append(packed)

    # W at half scale (exact for fp8 normals), TRN fp8 range
    w_half = (w_fp8fn.astype(np.float32) * 0.5).astype(E4M3FN)
    wqs = []
    for q in range(OF_WAYS):
        wq = w_half[q * OF_SH : (q + 1) * OF_SH]          # [OF_SH, DIN]
        w3 = np.ascontiguousarray(
            wq.T.reshape(KO, P, OF_SH).transpose(1, 0, 2)  # [P, KO, OF_SH]
        ).view(FP8_NP)
        wqs.append(w3)

    ws_arr = np.full((P, 1), ws_val, dtype=np.float32)

    in_maps = []
    for c in range(N_CORES):
        h, q = c // OF_WAYS, c % OF_WAYS
        in_maps.append(
            {
                "xh": halves[h],
                "xa": halves[h][q * AT : (q + 1) * AT],
                "w3": wqs[q],
                "ws": ws_arr,
            }
        )
    return in_maps


def _assemble(results):
    y = np.empty((TOKENS, DOUT), dtype=ml_dtypes.bfloat16)
    for c in range(N_CORES):
        h, q = c // OF_WAYS, c % OF_WAYS
        part = results[c]["y"]
        if part.dtype != ml_dtypes.bfloat16:
            part = part.view(ml_dtypes.bfloat16)
        y[h * T_SH : (h + 1) * T_SH, q * OF_SH : (q + 1) * OF_SH] = part
    return y


def kernel(x, weight, w_scale):
    nc = _build_module()
    in_maps = _pack_inputs(x, weight, w_scale)
    res = run_bass_kernel_spmd(nc, in_maps, list(range(N_CORES)))
    return _assemble(res.results)
